# revision 1
# baseline (speedup 1.0000x reference)
"""Quincunx ConvBlock (GN->ReLU->qcConv x2 + skip 1x1 conv + GN, residual add)
on 8 TRN2 NeuronCores.

Sharding: batch (2) x H-quarters (4) -> 8 cores. Each core owns 64 output rows
of both cosets. Cross-coset 2x2 taps handled with host-provided 1-2 row halos;
the three GroupNorms need global (batch, group) stats -> tiny AllReduces over
core groups {0..3} (batch 0) and {4..7} (batch 1).

SBUF layout: 128 partitions = [coset0 ch 0..63 | coset1 ch 0..63].
A conv then becomes, per 2-row output block (N=512), 1 full K=128 matmul
(center taps + the two (0,0)-shift corner taps) + 6 half K=64 matmuls (the
remaining corner taps) accumulated in PSUM.

Compute in bf16 (matmuls, activations storage), fp32 stats/PSUM.
"""
import numpy as np
import ml_dtypes

import concourse.bass as bass
import concourse.tile as tile
from concourse import bacc, mybir
from concourse.bass_utils import run_bass_kernel_spmd

BF16 = ml_dtypes.bfloat16

B, C, H, W = 2, 64, 256, 256
G = 8            # groups; 8 channels/group, stats joint over both cosets
EPS = 1e-5
NCORES = 8
RPC = H // 4     # 64 rows per core
XR = RPC + 4     # X tile rows (2 halo top, 2 halo bottom -> rows R0-2 .. R1+1)
HR = RPC + 2     # H tile rows (h rows R0-1 .. R1)
WP = W + 4       # padded row width; data at cc 2..258, pads cc1/cc259

DT = mybir.dt.float32
BF = mybir.dt.bfloat16
AF = mybir.ActivationFunctionType
ALU = mybir.AluOpType

_CACHE = {}
DEBUG = False


# --------------------------------------------------------------------------
# device kernel builder
# --------------------------------------------------------------------------
def _conv_block_mms(nc, psum, off, Xt, r0, c0, wpk, start_full, stop_last=True):
    """Emit the 7 matmuls of one 2-row qc-conv block.

    psum[*, off:off+512] accumulates out0 (partitions 0:64) / out1 (64:128)
    for output rows (i0, i0+1). (r0, c0) = rr/cc coords of the (0,0)-shift
    read. wpk = [128, 320] packed weights."""
    o = psum[:, off:off + 512]
    o0 = psum[0:64, off:off + 512]
    o1 = psum[64:128, off:off + 512]
    # full matmul, shift (0,0): both cosets
    nc.tensor.matmul(o, wpk[:, 0:128], Xt[:, r0:r0 + 2, c0:c0 + 256],
                     start=start_full, stop=False)
    # top halves: x0 shifted -> out1.  slots: (0,1), (1,0), (1,1)
    nc.tensor.matmul(o1, wpk[0:64, 128:192], Xt[0:64, r0:r0 + 2, c0 + 1:c0 + 257],
                     start=False, stop=False)
    nc.tensor.matmul(o1, wpk[0:64, 192:256], Xt[0:64, r0 + 1:r0 + 3, c0:c0 + 256],
                     start=False, stop=False)
    nc.tensor.matmul(o1, wpk[0:64, 256:320], Xt[0:64, r0 + 1:r0 + 3, c0 + 1:c0 + 257],
                     start=False, stop=False)
    # bottom halves: x1 shifted -> out0.  slots: (-1,-1), (-1,0), (0,-1)
    nc.tensor.matmul(o0, wpk[64:128, 128:192], Xt[64:128, r0 - 1:r0 + 1, c0 - 1:c0 + 255],
                     start=False, stop=False)
    nc.tensor.matmul(o0, wpk[64:128, 192:256], Xt[64:128, r0 - 1:r0 + 1, c0:c0 + 256],
                     start=False, stop=False)
    nc.tensor.matmul(o0, wpk[64:128, 256:320], Xt[64:128, r0:r0 + 2, c0 - 1:c0 + 255],
                     start=False, stop=stop_last)


def _rsqrt_newton(nc, out, v_ap, tmps):
    """out = rsqrt(v + EPS), accurate (ACT sqrt + DVE recip + 1 Newton step).
    tmps: dict of small [128,1] fp32 tiles."""
    ve, sd, y0, t, u, w = (tmps[k] for k in ("ve", "sd", "y0", "t", "u", "w"))
    nc.vector.tensor_scalar(ve[:], v_ap, EPS, None, ALU.add)
    nc.scalar.activation(sd[:], ve[:], AF.Sqrt)
    nc.vector.reciprocal(y0[:], sd[:])
    nc.vector.tensor_tensor(t[:], ve[:], y0[:], ALU.mult)
    nc.vector.tensor_tensor(u[:], t[:], y0[:], ALU.mult)
    nc.vector.tensor_scalar(w[:], u[:], -0.5, 1.5, ALU.mult, ALU.add)
    nc.vector.tensor_tensor(out, y0[:], w[:], ALU.mult)


def _build(core_is_top, core_is_bot):
    """Build the SPMD kernel. core_is_top/bot: whether THIS build handles
    boundary memsets -- but we build ONE program for all cores, so both
    boundary memsets are parameterized by a per-core input mask instead."""
    nc = bacc.Bacc("TRN2", target_bir_lowering=False, debug=False,
                   num_devices=NCORES)

    x_d = nc.dram_tensor("xin", [128, XR, WP], BF, kind="ExternalInput")
    cb_d = nc.dram_tensor("cbf", [128, 1024], BF, kind="ExternalInput")
    cf_d = nc.dram_tensor("cf32", [128, 17], DT, kind="ExternalInput")
    gt_d = nc.dram_tensor("gfull", [128, 128], DT, kind="ExternalInput")
    # per-core row-zero masks: [128, XR] and [128, HR] multipliers (1 or 0)
    mx_d = nc.dram_tensor("maskx", [128, XR], DT, kind="ExternalInput")
    mh_d = nc.dram_tensor("maskh", [128, HR], DT, kind="ExternalInput")
    out_d = nc.dram_tensor("out", [128, RPC, W], BF, kind="ExternalOutput")
    if DEBUG:
        dbg_d = nc.dram_tensor("dbg", [128, 20], DT, kind="ExternalOutput")
        dbgs_d = nc.dram_tensor("dbgs", [128, RPC, W], BF, kind="ExternalOutput")
        dbgh_d = nc.dram_tensor("dbgh", [128, HR, WP], BF, kind="ExternalOutput")

    with tile.TileContext(nc) as tc:
        with (
            tc.tile_pool(name="big", bufs=1) as big,
            tc.tile_pool(name="consts", bufs=1) as cpool,
            tc.tile_pool(name="stats", bufs=1) as spool,
            tc.tile_pool(name="trash", bufs=2) as tpool,
            tc.tile_pool(name="psum", bufs=2, space="PSUM") as pp,
            tc.tile_pool(name="psmall", bufs=1, space="PSUM") as ps,
            tc.tile_pool(name="dram", bufs=1, space="DRAM") as dp,
        ):
            Xt = big.tile([128, XR, WP], BF, tag="X")
            Hs = big.tile([128, HR, WP], BF, tag="H")
            S = big.tile([128, RPC, W], BF, tag="S")
            OUT = big.tile([128, RPC, W], BF, tag="OUT")

            cb = cpool.tile([128, 1024], BF)
            cf = cpool.tile([128, 17], DT)
            gt = cpool.tile([128, 128], DT)
            mx = cpool.tile([128, XR], DT)
            mh = cpool.tile([128, HR], DT)

            w1 = cb[:, 0:320]
            w2 = cb[:, 320:640]
            wsk = cb[:, 640:768]
            ident = cb[:, 768:896]
            gind_bf = cb[:, 896:1024]
            gind = cf[:, 0:8]
            pp_g1, pp_b1, pp_bias1 = cf[:, 8:9], cf[:, 9:10], cf[:, 10:11]
            pp_g2, pp_b2, pp_bias2 = cf[:, 11:12], cf[:, 12:13], cf[:, 13:14]
            pp_gs, pp_bs, pp_biass = cf[:, 14:15], cf[:, 15:16], cf[:, 16:17]

            # dummy collective first: absorbs the one-time CC barrier /
            # core-start skew while DMAs+stats run
            dmy = spool.tile([8, 1], DT, tag="dmy", name="dmy")
            dmy_i = dp.tile([8, 1], DT, tag="dmyi", name="dmy_i")
            dmy_o = dp.tile([8, 1], DT, tag="dmyo", name="dmy_o")
            nc.vector.memset(dmy[:], 0.0)
            nc.sync.dma_start(dmy_i[:], dmy[:])
            nc.gpsimd.collective_compute("AllReduce", ALU.add,
                                         replica_groups=[[0, 1, 2, 3], [4, 5, 6, 7]],
                                         ins=[dmy_i.opt()], outs=[dmy_o.opt()])

            # ---------------- DMAs in ----------------
            nc.sync.dma_start(cb[:], cb_d[:])
            nc.sync.dma_start(cf[:], cf_d[:])
            nc.sync.dma_start(gt[:], gt_d[:])
            nc.sync.dma_start(mx[:], mx_d[:])
            nc.sync.dma_start(mh[:], mh_d[:])
            for c in range(4):
                nc.sync.dma_start(Xt[:, 17 * c:17 * (c + 1), :],
                                  x_d[:, 17 * c:17 * (c + 1), :])

            # small fp32 stat tiles
            def st(name, shape, dt=DT):
                return spool.tile(shape, dt, tag=name, name=name)

            xsq = st("xsq", [128, 16])
            ssq = st("ssq", [128, 8])
            sdr = st("sdr", [128, 11])
            bn6h = st("bn6h", [128, RPC, 6])
            bnh = st("bnh", [128, 2])
            xm2 = st("xm2", [128, 1])
            sm1 = st("sm1", [128, 1])
            sm2 = st("sm2", [128, 1])
            stx = st("stx", [128, 2])
            sts = st("sts", [128, 2])
            sth = st("sth", [128, 2])
            g1b = st("g1b", [128, 2])
            gsb = st("gsb", [128, 2])
            g2b = st("g2b", [128, 2])
            a1, c1 = st("a1", [128, 1]), st("c1", [128, 1])
            a2, c2 = st("a2", [128, 1]), st("c2", [128, 1])
            asv, csv = st("asv", [128, 1]), st("csv", [128, 1])
            dcon = st("dcon", [128, 1])
            diag = st("diag", [128, 128], BF)
            tmps = {k: st("nt_" + k, [128, 1])
                    for k in ("ve", "sd", "y0", "t", "u", "w", "m", "mm")}

            ccx_i = dp.tile([128, 2], DT, tag="ccxi", name="ccx_i")
            ccx_o = dp.tile([128, 2], DT, tag="ccxo", name="ccx_o")
            ccs_i = dp.tile([128, 2], DT, tag="ccsi", name="ccs_i")
            ccs_o = dp.tile([128, 2], DT, tag="ccso", name="ccs_o")
            cch_i = dp.tile([128, 2], DT, tag="cchi", name="cch_i")
            cch_o = dp.tile([128, 2], DT, tag="ccho", name="cch_o")
            RG = [[0, 1, 2, 3], [4, 5, 6, 7]]

            # ---------------- phase 0: x stats (rows rr 2..66 = owned) ------
            # m2: ACT square chunks w/ accumulate; m1: PE group-indicator mms
            for j in range(16):
                tr = tpool.tile([128, 4, 256], BF, tag="trash")
                nc.scalar.activation(tr[:], Xt[:, 2 + 4 * j:6 + 4 * j, 2:258],
                                     AF.Square, accum_out=xsq[:, j:j + 1])
            pm1 = ps.tile([128, 512], DT, tag="pm1")
            for t in range(32):
                nc.tensor.matmul(pm1[:, :], gind_bf[:],
                                 Xt[:, 2 + 2 * t:4 + 2 * t, 2:258],
                                 start=(t == 0), stop=(t == 31),
                                 skip_group_check=True)

            # ---------------- skip conv + S drains (+ m1 accum) -------------
            sk_groups = [3] * 10 + [2]
            bi = 0
            for g, nb in enumerate(sk_groups):
                pg = pp.tile([128, 1536], DT, tag="pg")
                for k in range(nb):
                    t = bi + k
                    nc.tensor.matmul(pg[:, 512 * k:512 * (k + 1)], wsk[:],
                                     Xt[:, 2 + 2 * t:4 + 2 * t, 2:258],
                                     start=True, stop=True)
                nc.vector.tensor_scalar(S[:, 2 * bi:2 * (bi + nb), :],
                                        pg[:, 0:512 * nb], pp_biass, 0.0, ALU.add,
                                        ALU.add, accum_out=sdr[:, g:g + 1])
                bi += nb

            # ---------------- AR1: GN1 stats ----------------
            nc.vector.reduce_sum(xm2[:], xsq[:], axis=mybir.AxisListType.X)
            pstat = ps.tile([128, 512], DT, tag="pstat", name="pstat")
            nc.tensor.matmul(pstat[:, 0:1], gt[:], xm2[:], start=True, stop=True)
            nc.vector.reduce_sum(stx[:, 0:1], pm1[:, :], axis=mybir.AxisListType.X)
            nc.vector.tensor_copy(stx[:, 1:2], pstat[:, 0:1])
            if DEBUG:
                dbg_pre = spool.tile([128, 2], DT, tag="dbg_pre", name="dbg_pre")
                dbg_post = spool.tile([128, 2], DT, tag="dbg_post", name="dbg_post")
                nc.vector.tensor_copy(dbg_pre[:], stx[:])
            nc.sync.dma_start(ccx_i[:], stx[:])
            nc.gpsimd.collective_compute("AllReduce", ALU.add, replica_groups=RG,
                                         ins=[ccx_i.opt()], outs=[ccx_o.opt()])
            nc.sync.dma_start(stx[:], ccx_o[:])
            if DEBUG:
                nc.vector.tensor_copy(dbg_post[:], stx[:])
            nc.vector.tensor_scalar(g1b[:], stx[:], 1.0 / (1 << 20), None, ALU.mult)
            # var = E2 - mean^2 ; a1 = g1 * rsqrt(var+eps); c1 = b1 - mean*a1
            nc.vector.tensor_tensor(tmps["m"][:], g1b[:, 0:1], g1b[:, 0:1], ALU.mult)
            nc.vector.tensor_tensor(tmps["mm"][:], g1b[:, 1:2], tmps["m"][:], ALU.subtract)
            _rsqrt_newton(nc, tmps["sd"][:], tmps["mm"][:], tmps)  # reuse sd as rsqrt out
            nc.vector.tensor_tensor(a1[:], tmps["sd"][:], pp_g1, ALU.mult)
            nc.vector.tensor_tensor(tmps["m"][:], g1b[:, 0:1], a1[:], ALU.mult)
            nc.vector.tensor_tensor(c1[:], pp_b1, tmps["m"][:], ALU.subtract)

            # ---------------- apply1: X <- relu(a1*X + c1), then mask -------
            for c in range(6):
                r0, r1 = 12 * c, min(12 * (c + 1), XR)
                nc.scalar.activation(Xt[:, r0:r1, 2:258], Xt[:, r0:r1, 2:258],
                                     AF.Relu, bias=c1[:], scale=a1[:])
            # zero out-of-range halo rows (core 0 / core 3): multiply by mask
            for r in (0, 1, XR - 1):
                nc.vector.tensor_scalar(Xt[:, r:r + 1, 2:258], Xt[:, r:r + 1, 2:258],
                                        mx[:, r:r + 1], None, ALU.mult)

            # H pad columns must be zero before conv2 reads them
            nc.gpsimd.memset(Hs[:, :, 1:2], 0.0)
            nc.gpsimd.memset(Hs[:, :, 258:259], 0.0)

            # ---------------- conv1 + H drains + bn_h -----------------------
            # output rows i0 = R0-1+2t, t=0..32 ; X (0,0)-read at rr=1+2t, cc=1
            bn_emitted = 0
            for g in range(11):
                pg = pp.tile([128, 1536], DT, tag="pg")
                for k in range(3):
                    t = 3 * g + k
                    _conv_block_mms(nc, pg, 512 * k, Xt, 1 + 2 * t, 2, w1, True)
                nc.vector.tensor_scalar(Hs[:, 6 * g:6 * g + 6, 2:258],
                                        pg[:, 0:1536], pp_bias1, None, ALU.add)
                # bn_h rows (walrus: one 6-elem stats window per op)
                while bn_emitted < 64 and 2 + bn_emitted <= 6 * g + 6:
                    j = bn_emitted
                    nc.vector.bn_stats(bn6h[:, j:j + 1, :],
                                       Hs[:, 1 + j:2 + j, 2:258])
                    bn_emitted += 1

            # ---------------- s stats (m2 squares) + AR1b -------------------
            for j in range(8):
                trg = tpool.tile([128, 8, 256], BF, tag="trashg", name="trg")
                nc.scalar.activation(trg[:], S[:, 8 * j:8 * (j + 1), :],
                                     AF.Square, accum_out=ssq[:, j:j + 1])
            nc.vector.reduce_sum(sm1[:], sdr[:], axis=mybir.AxisListType.X)
            nc.vector.reduce_sum(sm2[:], ssq[:], axis=mybir.AxisListType.X)
            nc.tensor.matmul(pstat[:, 8:9], gt[:], sm1[:], start=True, stop=True)
            nc.tensor.matmul(pstat[:, 9:10], gt[:], sm2[:], start=True, stop=True)
            nc.vector.tensor_copy(sts[:], pstat[:, 8:10])
            nc.sync.dma_start(ccs_i[:], sts[:])
            nc.gpsimd.collective_compute("AllReduce", ALU.add, replica_groups=RG,
                                         ins=[ccs_i.opt()], outs=[ccs_o.opt()])
            nc.sync.dma_start(sts[:], ccs_o[:])
            nc.vector.tensor_scalar(gsb[:], sts[:], 1.0 / (1 << 20), None, ALU.mult)
            nc.vector.tensor_tensor(tmps["m"][:], gsb[:, 0:1], gsb[:, 0:1], ALU.mult)
            nc.vector.tensor_tensor(tmps["mm"][:], gsb[:, 1:2], tmps["m"][:], ALU.subtract)
            _rsqrt_newton(nc, tmps["sd"][:], tmps["mm"][:], tmps)
            nc.vector.tensor_tensor(asv[:], tmps["sd"][:], pp_gs, ALU.mult)
            nc.vector.tensor_tensor(tmps["m"][:], gsb[:, 0:1], asv[:], ALU.mult)
            nc.vector.tensor_tensor(csv[:], pp_bs, tmps["m"][:], ALU.subtract)
            # diag(a_s) for the PE-side residual add; drain const = bias2 + c_s
            nc.vector.tensor_scalar(diag[:], ident[:], asv[:], None, ALU.mult)
            nc.vector.tensor_tensor(dcon[:], pp_bias2, csv[:], ALU.add)

            # ---------------- AR2: GN2 stats --------------------------------
            nc.vector.bn_aggr(bnh[:], bn6h[:])
            # t2h = (mean, E2) per partition
            nc.vector.tensor_tensor(tmps["m"][:], bnh[:, 0:1], bnh[:, 0:1], ALU.mult)
            nc.vector.tensor_tensor(tmps["mm"][:], bnh[:, 1:2], tmps["m"][:], ALU.add)
            nc.tensor.matmul(pstat[:, 16:17], gt[:], bnh[:, 0:1], start=True, stop=True)
            nc.tensor.matmul(pstat[:, 17:18], gt[:], tmps["mm"][:], start=True, stop=True)
            nc.vector.tensor_copy(sth[:], pstat[:, 16:18])
            nc.sync.dma_start(cch_i[:], sth[:])
            nc.gpsimd.collective_compute("AllReduce", ALU.add, replica_groups=RG,
                                         ins=[cch_i.opt()], outs=[cch_o.opt()])
            nc.sync.dma_start(sth[:], cch_o[:])
            nc.vector.tensor_scalar(g2b[:], sth[:], 1.0 / 64, None, ALU.mult)
            nc.vector.tensor_tensor(tmps["m"][:], g2b[:, 0:1], g2b[:, 0:1], ALU.mult)
            nc.vector.tensor_tensor(tmps["mm"][:], g2b[:, 1:2], tmps["m"][:], ALU.subtract)
            _rsqrt_newton(nc, tmps["sd"][:], tmps["mm"][:], tmps)
            nc.vector.tensor_tensor(a2[:], tmps["sd"][:], pp_g2, ALU.mult)
            nc.vector.tensor_tensor(tmps["m"][:], g2b[:, 0:1], a2[:], ALU.mult)
            nc.vector.tensor_tensor(c2[:], pp_b2, tmps["m"][:], ALU.subtract)

            # ---------------- apply2: H <- relu(a2*H + c2), mask ------------
            for c in range(6):
                r0, r1 = 11 * c, min(11 * (c + 1), HR)
                nc.scalar.activation(Hs[:, r0:r1, 2:258], Hs[:, r0:r1, 2:258],
                                     AF.Relu, bias=c2[:], scale=a2[:])
            for r in (0, HR - 1):
                nc.vector.tensor_scalar(Hs[:, r:r + 1, 2:258], Hs[:, r:r + 1, 2:258],
                                        mh[:, r:r + 1], None, ALU.mult)

            # ---------------- conv2 + residual + OUT drains + DMA out ------
            # output rows i0 = R0+2t, t=0..31 ; H (0,0)-read at rr=1+2t, cc=1
            c2_groups = [3] * 10 + [2]
            bi = 0
            dma_done = 0
            for g, nb in enumerate(c2_groups):
                pg = pp.tile([128, 1536], DT, tag="pg")
                for k in range(nb):
                    t = bi + k
                    _conv_block_mms(nc, pg, 512 * k, Hs, 1 + 2 * t, 2, w2, True,
                                    stop_last=False)
                    # residual: psum += diag(a_s) @ s  (accumulate, ends group)
                    nc.tensor.matmul(pg[:, 512 * k:512 * (k + 1)], diag[:],
                                     S[:, 2 * t:2 * t + 2, :],
                                     start=False, stop=True)
                nc.vector.tensor_scalar(OUT[:, 2 * bi:2 * (bi + nb), :],
                                        pg[:, 0:512 * nb], dcon, None, ALU.add)
                bi += nb
                while dma_done < 4 and 16 * (dma_done + 1) <= 2 * bi:
                    c = dma_done
                    nc.sync.dma_start(out_d[:, 16 * c:16 * (c + 1), :],
                                      OUT[:, 16 * c:16 * (c + 1), :])
                    dma_done += 1

            if DEBUG:
                dbg = spool.tile([128, 20], DT, tag="dbg", name="dbg")
                for i, src_ap in enumerate((a1, c1, a2, c2, asv, csv, dcon,
                                            xm2, sm1, sm2)):
                    nc.vector.tensor_copy(dbg[:, i:i + 1], src_ap[:])
                nc.vector.tensor_copy(dbg[:, 10:12], g1b[:])
                nc.vector.tensor_copy(dbg[:, 12:14], gsb[:])
                nc.vector.tensor_copy(dbg[:, 14:16], g2b[:])
                nc.vector.tensor_copy(dbg[:, 16:18], dbg_pre[:])
                nc.vector.tensor_copy(dbg[:, 18:20], dbg_post[:])
                nc.sync.dma_start(dbg_d[:], dbg[:])
                nc.sync.dma_start(dbgs_d[:], S[:])
                nc.sync.dma_start(dbgh_d[:], Hs[:])

    nc.compile()
    return nc


def _get_nc():
    if "nc" not in _CACHE:
        _CACHE["nc"] = _build(None, None)
    return _CACHE["nc"]


# --------------------------------------------------------------------------
# host side
# --------------------------------------------------------------------------
def _pack_weights(w_center, w_corner):
    """[128, 320] packed lhsT weights for one qc conv."""
    wp = np.zeros((128, 320), np.float32)
    wc = w_center.T
    wk = lambda a, b: w_corner[:, :, a, b].T
    wp[0:64, 0:64] = wc
    wp[0:64, 64:128] = wk(0, 0)
    wp[64:128, 0:64] = wk(1, 1)
    wp[64:128, 64:128] = wc
    # top half slots (x0 -> out1): shifts (0,1),(1,0),(1,1)
    wp[0:64, 128:192] = wk(0, 1)
    wp[0:64, 192:256] = wk(1, 0)
    wp[0:64, 256:320] = wk(1, 1)
    # bottom half slots (x1 -> out0): shifts (-1,-1),(-1,0),(0,-1) = taps (0,0),(0,1),(1,0)
    wp[64:128, 128:192] = wk(0, 0)
    wp[64:128, 192:256] = wk(0, 1)
    wp[64:128, 256:320] = wk(1, 0)
    return wp


def kernel(x0, x1, g1, b1, w1_center, w1_corner, bias1,
           g2, b2, w2_center, w2_corner, bias2,
           w_skip, bias_skip, g_skip, beta_skip):
    x0 = np.asarray(x0, np.float32)
    x1 = np.asarray(x1, np.float32)

    # ---- constants ----
    cbf = np.zeros((128, 1024), np.float32)
    cbf[:, 0:320] = _pack_weights(np.asarray(w1_center), np.asarray(w1_corner))
    cbf[:, 320:640] = _pack_weights(np.asarray(w2_center), np.asarray(w2_corner))
    wskf = np.zeros((128, 128), np.float32)
    wskf[0:64, 0:64] = np.asarray(w_skip).T
    wskf[64:128, 64:128] = np.asarray(w_skip).T
    cbf[:, 640:768] = wskf
    cbf[:, 768:896] = np.eye(128, dtype=np.float32)
    gind = np.zeros((128, 8), np.float32)
    for p in range(128):
        gind[p, (p % 64) // 8] = 1.0
    gfull = gind @ gind.T
    cbf[:, 896:1024] = gfull
    cbf = cbf.astype(BF16)

    cf32 = np.zeros((128, 17), np.float32)
    pp2 = lambda v: np.concatenate([np.asarray(v, np.float32)] * 2)
    for i, v in enumerate((g1, b1, bias1, g2, b2, bias2, g_skip, beta_skip, bias_skip)):
        cf32[:, 8 + i] = pp2(v)
    gindt = gfull

    # ---- per-core inputs ----
    in_maps = []
    for core in range(NCORES):
        b, k = core // 4, core % 4
        r0 = RPC * k
        xc = np.zeros((128, XR, WP), np.float32)
        lo, hi = r0 - 2, r0 + RPC + 2   # rows RB .. R1+1 inclusive -> [lo, hi)
        vlo, vhi = max(0, lo), min(H, hi)
        xc[0:64, vlo - lo:vhi - lo, 2:258] = x0[b, :, vlo:vhi, :]
        xc[64:128, vlo - lo:vhi - lo, 2:258] = x1[b, :, vlo:vhi, :]

        maskx = np.ones((128, XR), np.float32)
        if k == 0:
            maskx[:, 0:2] = 0.0
        if k == 3:
            maskx[:, XR - 1] = 0.0
        maskh = np.ones((128, HR), np.float32)
        if k == 0:
            maskh[64:128, 0] = 0.0     # h1[-1] must stay zero
        if k == 3:
            maskh[0:64, HR - 1] = 0.0  # h0[H] must stay zero

        in_maps.append({
            "xin": xc.astype(BF16),
            "cbf": cbf, "cf32": cf32, "gfull": gindt,
            "maskx": maskx, "maskh": maskh,
        })

    nc = _get_nc()
    _CACHE["in_maps"] = in_maps
    res = run_bass_kernel_spmd(nc, in_maps, list(range(NCORES)))
    _CACHE["last_results"] = res

    out = np.empty((2, B, C, H, W), np.float32)
    for core in range(NCORES):
        b, k = core // 4, core % 4
        r0 = RPC * k
        arr = np.asarray(res.results[core]["out"]).astype(np.float32)
        out[0, b, :, r0:r0 + RPC, :] = arr[0:64]
        out[1, b, :, r0:r0 + RPC, :] = arr[64:128]
    return out



# revision 2
# speedup vs baseline: 1.1107x; 1.1107x over previous
"""Quincunx ConvBlock (GN->ReLU->qcConv x2 + skip 1x1 conv + GN, residual add)
on 8 TRN2 NeuronCores.

Sharding: batch (2) x H-quarters (4) -> 8 cores. Each core owns 64 output rows
of both cosets. GroupNorm stats need global (batch, group) sums -> two small
AllReduces over core groups {0..3} / {4..7} (GN1; skip-GN + GN2 merged).

SBUF layout: 128 partitions = [coset0 ch 0..63 | coset1 ch 0..63].
Conv tiles store coset1 pre-shifted by (+1,+1) so each 2-row conv block is
4 full K=128 matmuls (one per 2x2 tap shift); the center taps fold into the
(0,0)/(1,1) shift weights. Plain-layout tiles feed stats and the skip conv.

PSUM drains run on the Scalar (ACT) engine with sum-accumulators harvesting
GN stats; square-sums via DVE scalar_tensor_tensor. Applies (ReLU) interleave
with conv matmul groups so ACT and PE overlap.
"""
import numpy as np
import ml_dtypes

import concourse.bass as bass
import concourse.tile as tile
from concourse import bacc, mybir
from concourse.bass_utils import run_bass_kernel_spmd

BF16 = ml_dtypes.bfloat16

B, C, H, W = 2, 64, 256, 256
G = 8
EPS = 1e-5
NCORES = 8
RPC = H // 4     # 64 owned rows per core
XR = RPC + 4     # conv x tile rows
HR = RPC + 2     # h tile rows (h rows R0-1 .. R1+1)
WP = W + 4       # padded row width for conv tiles

DT = mybir.dt.float32
BF = mybir.dt.bfloat16
AF = mybir.ActivationFunctionType
ALU = mybir.AluOpType

SHIFTS = ((0, 0), (0, 1), (1, 0), (1, 1))

_CACHE = {}


def _rsqrt_newton(nc, out, v_ap, tmps):
    """out = rsqrt(v + EPS) via ACT sqrt + DVE recip + 1 Newton step."""
    ve, sd, y0, t, u, w = (tmps[k] for k in ("ve", "sd", "y0", "t", "u", "w"))
    nc.vector.tensor_scalar(ve[:], v_ap, EPS, None, ALU.add)
    nc.scalar.activation(sd[:], ve[:], AF.Sqrt)
    nc.vector.reciprocal(y0[:], sd[:])
    nc.vector.tensor_tensor(t[:], ve[:], y0[:], ALU.mult)
    nc.vector.tensor_tensor(u[:], t[:], y0[:], ALU.mult)
    nc.vector.tensor_scalar(w[:], u[:], -0.5, 1.5, ALU.mult, ALU.add)
    nc.vector.tensor_tensor(out, y0[:], w[:], ALU.mult)


def _gn_coeffs(nc, a, c, mean_ap, e2_ap, g_ap, b_ap, tmps):
    """a = g*rsqrt(var+eps), c = b - mean*a  from (mean, E[x^2])."""
    nc.vector.tensor_tensor(tmps["m"][:], mean_ap, mean_ap, ALU.mult)
    nc.vector.tensor_tensor(tmps["mm"][:], e2_ap, tmps["m"][:], ALU.subtract)
    _rsqrt_newton(nc, tmps["sd"][:], tmps["mm"][:], tmps)
    nc.vector.tensor_tensor(a[:], tmps["sd"][:], g_ap, ALU.mult)
    nc.vector.tensor_tensor(tmps["m"][:], mean_ap, a[:], ALU.mult)
    nc.vector.tensor_tensor(c[:], b_ap, tmps["m"][:], ALU.subtract)


def _build():
    nc = bacc.Bacc("TRN2", target_bir_lowering=False, debug=False,
                   num_devices=NCORES)

    xp_d = nc.dram_tensor("xp", [128, RPC, W], BF, kind="ExternalInput")
    x2_d = nc.dram_tensor("x2", [128, XR, WP], BF, kind="ExternalInput")
    cb_d = nc.dram_tensor("cbf", [128, 1408], BF, kind="ExternalInput")
    cf_d = nc.dram_tensor("cf32", [128, 16], DT, kind="ExternalInput")
    gt_d = nc.dram_tensor("gfull", [128, 128], DT, kind="ExternalInput")
    out_d = nc.dram_tensor("out", [128, RPC, W], BF, kind="ExternalOutput")

    with tile.TileContext(nc) as tc:
        with (
            tc.tile_pool(name="big", bufs=1) as big,
            tc.tile_pool(name="consts", bufs=1) as cpool,
            tc.tile_pool(name="stats", bufs=1) as spool,
            tc.tile_pool(name="trash", bufs=2) as tpool,
            tc.tile_pool(name="psum", bufs=2, space="PSUM") as pp,
            tc.tile_pool(name="psmall", bufs=1, space="PSUM") as ps,
            tc.tile_pool(name="dram", bufs=1, space="DRAM") as dp,
        ):
            X2 = big.tile([128, XR, WP], BF, tag="X2")
            XO = big.tile([128, RPC, W], BF, tag="XO")   # plain x, later OUT
            Hs = big.tile([128, HR, W], BF, tag="Hs")    # unshifted h
            H2 = big.tile([128, HR, WP], BF, tag="H2")   # conv-layout h
            S = big.tile([128, RPC, W], BF, tag="S")

            cb = cpool.tile([128, 1408], BF)
            cf = cpool.tile([128, 16], DT)
            gt = cpool.tile([128, 128], DT)

            w1 = cb[:, 0:512]
            w2 = cb[:, 512:1024]
            wsk = cb[:, 1024:1152]
            ident = cb[:, 1152:1280]
            gind_bf = cb[:, 1280:1408]
            pp_g1, pp_b1, pp_bias1 = cf[:, 0:1], cf[:, 1:2], cf[:, 2:3]
            pp_g2, pp_b2, pp_bias2 = cf[:, 3:4], cf[:, 4:5], cf[:, 5:6]
            pp_gs, pp_bs, pp_biass = cf[:, 6:7], cf[:, 7:8], cf[:, 8:9]
            mxa, mxb = cf[:, 9:10], cf[:, 10:11]
            mha, mhb = cf[:, 11:12], cf[:, 12:13]

            # dummy collective: absorbs CC barrier / core-start skew
            dmy = spool.tile([8, 1], DT, tag="dmy", name="dmy")
            dmy_i = dp.tile([8, 1], DT, tag="dmyi", name="dmy_i")
            dmy_o = dp.tile([8, 1], DT, tag="dmyo", name="dmy_o")
            nc.vector.memset(dmy[:], 0.0)
            nc.sync.dma_start(dmy_i[:], dmy[:])
            RG = [[0, 1, 2, 3], [4, 5, 6, 7]]
            nc.gpsimd.collective_compute("AllReduce", ALU.add,
                                         replica_groups=RG,
                                         ins=[dmy_i.opt()], outs=[dmy_o.opt()])

            # ---------------- DMAs in ----------------
            nc.sync.dma_start(cb[:], cb_d[:])
            nc.sync.dma_start(cf[:], cf_d[:])
            nc.sync.dma_start(gt[:], gt_d[:])
            for c in range(8):
                nc.sync.dma_start(XO[:, 8 * c:8 * (c + 1), :],
                                  xp_d[:, 8 * c:8 * (c + 1), :])
            for c in range(4):
                nc.sync.dma_start(X2[:, 17 * c:17 * (c + 1), :],
                                  x2_d[:, 17 * c:17 * (c + 1), :])

            def st(name, shape, dt=DT):
                return spool.tile(shape, dt, tag=name, name=name)

            xsq = st("xsq", [128, 8])
            sdr = st("sdr", [128, 11])
            ssq = st("ssq", [128, 11])
            hdr = st("hdr", [128, 11])
            hsq = st("hsq", [128, 11])
            xm2 = st("xm2", [128, 1])
            stx = st("stx", [128, 2])
            st4 = st("st4", [128, 4])
            s4r = st("s4r", [128, 4])
            g1b = st("g1b", [128, 2])
            g4b = st("g4b", [128, 4])
            a1, c1 = st("a1", [128, 1]), st("c1", [128, 1])
            a2, c2 = st("a2", [128, 1]), st("c2", [128, 1])
            asv, csv = st("asv", [128, 1]), st("csv", [128, 1])
            dcon = st("dcon", [128, 1])
            diag = st("diag", [128, 128], BF)
            tmps = {k: st("nt_" + k, [128, 1])
                    for k in ("ve", "sd", "y0", "t", "u", "w", "m", "mm")}

            ccx_i = dp.tile([128, 2], DT, tag="ccxi", name="ccx_i")
            ccx_o = dp.tile([128, 2], DT, tag="ccxo", name="ccx_o")
            cc4_i = dp.tile([128, 4], DT, tag="cc4i", name="cc4_i")
            cc4_o = dp.tile([128, 4], DT, tag="cc4o", name="cc4_o")

            # H2 pad columns (never written by drain copies) must be zero
            nc.gpsimd.memset(H2[0:64, :, 258:259], 0.0)
            nc.gpsimd.memset(H2[64:128, :, 2:3], 0.0)

            # ---------------- GN1 stats (ACT squares + PE sums) -------------
            for j in range(8):
                tr = tpool.tile([128, 8, 256], BF, tag="trash")
                nc.scalar.activation(tr[:], XO[:, 8 * j:8 * (j + 1), :],
                                     AF.Square, accum_out=xsq[:, j:j + 1])
            pm1 = ps.tile([128, 512], DT, tag="pm1")
            for t in range(32):
                nc.tensor.matmul(pm1[:, :], gind_bf[:],
                                 XO[:, 2 * t:2 * t + 2, :],
                                 start=(t == 0), stop=(t == 31),
                                 skip_group_check=True)

            # ---------------- AR1 ----------------
            nc.vector.reduce_sum(xm2[:], xsq[:], axis=mybir.AxisListType.X)
            pstat = ps.tile([128, 512], DT, tag="pstat", name="pstat")
            nc.tensor.matmul(pstat[:, 0:1], gt[:], xm2[:], start=True, stop=True)
            nc.vector.reduce_sum(stx[:, 0:1], pm1[:, :], axis=mybir.AxisListType.X)
            nc.vector.tensor_copy(stx[:, 1:2], pstat[:, 0:1])
            nc.sync.dma_start(ccx_i[:], stx[:])
            nc.gpsimd.collective_compute("AllReduce", ALU.add, replica_groups=RG,
                                         ins=[ccx_i.opt()], outs=[ccx_o.opt()])
            nc.sync.dma_start(stx[:], ccx_o[:])

            # ---------------- skip conv + S drains (DVE) + ssq --------------
            sk_groups = [3] * 10 + [2]
            bi = 0
            for g, nb in enumerate(sk_groups):
                pg = pp.tile([128, 1536], DT, tag="pg")
                for k in range(nb):
                    t = bi + k
                    nc.tensor.matmul(pg[:, 512 * k:512 * (k + 1)], wsk[:],
                                     XO[:, 2 * t:2 * t + 2, :],
                                     start=True, stop=True)
                nc.vector.tensor_scalar(S[:, 2 * bi:2 * (bi + nb), :],
                                        pg[:, 0:512 * nb], pp_biass, 0.0, ALU.add,
                                        ALU.add, accum_out=sdr[:, g:g + 1])
                trs = tpool.tile([128, 6, 256], BF, tag="trs", name="trs")
                nc.vector.scalar_tensor_tensor(
                    trs[:, 0:nb * 2, :], S[:, 2 * bi:2 * (bi + nb), :], 1.0,
                    S[:, 2 * bi:2 * (bi + nb), :], ALU.mult, ALU.mult,
                    accum_out=ssq[:, g:g + 1])
                bi += nb
                if g == 4:
                    # GN1 coeffs (DVE): placed mid-queue so the ops are ready
                    # right as the AR1 result lands
                    nc.vector.tensor_scalar(g1b[:], stx[:], 1.0 / (1 << 20),
                                            None, ALU.mult)
                    _gn_coeffs(nc, a1, c1, g1b[:, 0:1], g1b[:, 1:2],
                               pp_g1, pp_b1, tmps)

            # ---------------- apply1 on X2 (ACT) ----------------------------
            # slivers: the one data column outside the shared range per half
            nc.scalar.activation(X2[0:64, :, 2:3], X2[0:64, :, 2:3],
                                 AF.Relu, bias=c1[0:64], scale=a1[0:64])
            nc.scalar.activation(X2[64:128, :, 258:259], X2[64:128, :, 258:259],
                                 AF.Relu, bias=c1[64:128], scale=a1[64:128])

            a1_chunks = [(0, 12), (12, 24), (24, 36), (36, 48), (48, 60), (60, 68)]

            def emit_apply1(c):
                r0, r1 = a1_chunks[c]
                nc.scalar.activation(X2[:, r0:r1, 3:258], X2[:, r0:r1, 3:258],
                                     AF.Relu, bias=c1[:], scale=a1[:])
                if c == 0:  # row 2 mask (x1 row -1 on core 0)
                    nc.vector.tensor_scalar(X2[:, 2:3, 2:259], X2[:, 2:3, 2:259],
                                            mxa, None, ALU.mult)
                if c == 5:  # row 66 mask (x0 row H on core 3)
                    nc.vector.tensor_scalar(X2[:, 66:67, 2:259], X2[:, 66:67, 2:259],
                                            mxb, None, ALU.mult)

            emit_apply1(0)
            emit_apply1(1)

            # ---------------- conv1 + drains (ACT) + H2 DMAs + hsq ----------
            def conv_block(pg, off, T, r0, wpk, extra=None):
                for si, (dr, dc) in enumerate(SHIFTS):
                    nc.tensor.matmul(pg[:, off:off + 512],
                                     wpk[:, 128 * si:128 * (si + 1)],
                                     T[:, r0 + dr:r0 + dr + 2, 2 + dc:2 + dc + 256],
                                     start=(si == 0),
                                     stop=(si == 3 and extra is None))
                if extra is not None:
                    lhs, rhs = extra
                    nc.tensor.matmul(pg[:, off:off + 512], lhs, rhs,
                                     start=False, stop=True)

            napply = 2
            for g in range(11):
                pg = pp.tile([128, 1536], DT, tag="pg")
                for k in range(3):
                    t = 3 * g + k
                    conv_block(pg, 512 * k, X2, 1 + 2 * t, w1)
                # drain rows 6g..6g+5 to Hs on ACT; accumulate sums over
                # owned h rows only (exclude storage rows 0 and 65)
                if g == 0:
                    nc.scalar.activation(Hs[:, 0:1, :], pg[:, 0:256],
                                         AF.Identity, bias=pp_bias1)
                    nc.scalar.activation(Hs[:, 1:6, :], pg[:, 256:1536],
                                         AF.Identity, bias=pp_bias1,
                                         accum_out=hdr[:, 0:1])
                elif g == 10:
                    nc.scalar.activation(Hs[:, 60:65, :], pg[:, 0:1280],
                                         AF.Identity, bias=pp_bias1,
                                         accum_out=hdr[:, 10:11])
                    nc.scalar.activation(Hs[:, 65:66, :], pg[:, 1280:1536],
                                         AF.Identity, bias=pp_bias1)
                else:
                    nc.scalar.activation(Hs[:, 6 * g:6 * g + 6, :], pg[:, 0:1536],
                                         AF.Identity, bias=pp_bias1,
                                         accum_out=hdr[:, g:g + 1])
                # interleave remaining apply1 chunks between drains
                if napply < 6:
                    emit_apply1(napply)
                    napply += 1
                # conv-layout copies (DMA): h0 straight, h1 shifted (+1,+1)
                nc.sync.dma_start(H2[0:64, 6 * g:6 * g + 6, 2:258],
                                  Hs[0:64, 6 * g:6 * g + 6, :])
                if g < 10:
                    nc.sync.dma_start(H2[64:128, 6 * g + 1:6 * g + 7, 3:259],
                                      Hs[64:128, 6 * g:6 * g + 6, :])
                else:
                    nc.sync.dma_start(H2[64:128, 61:66, 3:259],
                                      Hs[64:128, 60:65, :])
                # sum of h^2 over owned rows (DVE)
                ra, rb = (1, 6) if g == 0 else (6 * g, min(6 * g + 6, 65))
                trh = tpool.tile([128, 6, 256], BF, tag="trh", name="trh")
                nc.vector.scalar_tensor_tensor(
                    trh[:, 0:rb - ra, :], Hs[:, ra:rb, :], 1.0,
                    Hs[:, ra:rb, :], ALU.mult, ALU.mult,
                    accum_out=hsq[:, g:g + 1])

            # ---------------- AR2 (skip-GN + GN2 merged) --------------------
            nc.vector.reduce_sum(st4[:, 0:1], sdr[:], axis=mybir.AxisListType.X)
            nc.vector.reduce_sum(st4[:, 1:2], ssq[:], axis=mybir.AxisListType.X)
            nc.vector.reduce_sum(st4[:, 2:3], hdr[:], axis=mybir.AxisListType.X)
            nc.vector.reduce_sum(st4[:, 3:4], hsq[:], axis=mybir.AxisListType.X)
            nc.tensor.matmul(pstat[:, 8:12], gt[:], st4[:], start=True, stop=True)
            nc.vector.tensor_copy(s4r[:], pstat[:, 8:12])
            nc.sync.dma_start(cc4_i[:], s4r[:])
            nc.gpsimd.collective_compute("AllReduce", ALU.add, replica_groups=RG,
                                         ins=[cc4_i.opt()], outs=[cc4_o.opt()])
            nc.sync.dma_start(s4r[:], cc4_o[:])
            nc.vector.tensor_scalar(g4b[:], s4r[:], 1.0 / (1 << 20), None, ALU.mult)
            _gn_coeffs(nc, asv, csv, g4b[:, 0:1], g4b[:, 1:2], pp_gs, pp_bs, tmps)
            _gn_coeffs(nc, a2, c2, g4b[:, 2:3], g4b[:, 3:4], pp_g2, pp_b2, tmps)
            nc.vector.tensor_scalar(diag[:], ident[:], asv[:], None, ALU.mult)
            nc.vector.tensor_tensor(dcon[:], pp_bias2, csv[:], ALU.add)

            # ---------------- apply2 on H2 (ACT) ----------------------------
            nc.scalar.activation(H2[0:64, :, 2:3], H2[0:64, :, 2:3],
                                 AF.Relu, bias=c2[0:64], scale=a2[0:64])
            nc.scalar.activation(H2[64:128, :, 258:259], H2[64:128, :, 258:259],
                                 AF.Relu, bias=c2[64:128], scale=a2[64:128])

            a2_chunks = [(0, 11), (11, 22), (22, 33), (33, 44), (44, 55), (55, 66)]

            def emit_apply2(c):
                r0, r1 = a2_chunks[c]
                nc.scalar.activation(H2[:, r0:r1, 3:258], H2[:, r0:r1, 3:258],
                                     AF.Relu, bias=c2[:], scale=a2[:])
                if c == 0:  # row 1 mask (h1 row -1 on core 0)
                    nc.vector.tensor_scalar(H2[:, 1:2, 2:259], H2[:, 1:2, 2:259],
                                            mha, None, ALU.mult)
                if c == 5:  # row 65 mask (h0 row H on core 3)
                    nc.vector.tensor_scalar(H2[:, 65:66, 2:259], H2[:, 65:66, 2:259],
                                            mhb, None, ALU.mult)

            emit_apply2(0)
            emit_apply2(1)

            # ---------------- conv2 + residual + OUT drains + DMA out -------
            c2_groups = [3] * 10 + [2]
            bi = 0
            napply = 2
            for g, nb in enumerate(c2_groups):
                pg = pp.tile([128, 1536], DT, tag="pg")
                for k in range(nb):
                    t = bi + k
                    conv_block(pg, 512 * k, H2, 1 + 2 * t, w2,
                               extra=(diag[:], S[:, 2 * t:2 * t + 2, :]))
                nc.scalar.activation(XO[:, 2 * bi:2 * (bi + nb), :],
                                     pg[:, 0:512 * nb], AF.Identity, bias=dcon[:])
                if napply < 6:
                    emit_apply2(napply)
                    napply += 1
                nc.sync.dma_start(out_d[:, 2 * bi:2 * (bi + nb), :],
                                  XO[:, 2 * bi:2 * (bi + nb), :])
                bi += nb

    nc.compile()
    return nc


def _get_nc():
    if "nc" not in _CACHE:
        _CACHE["nc"] = _build()
    return _CACHE["nc"]


# --------------------------------------------------------------------------
# host side
# --------------------------------------------------------------------------
def _pack_weights(w_center, w_corner):
    """[128, 512] packed lhsT weights: 4 shift blocks of [128,128]."""
    wp = np.zeros((128, 512), np.float32)
    wc = w_center.T
    wk = lambda a, b: w_corner[:, :, a, b].T
    for si, (a, b) in enumerate(SHIFTS):
        blk = wp[:, 128 * si:128 * (si + 1)]
        blk[0:64, 64:128] = wk(a, b)    # x0 -> out1, tap (a,b)
        blk[64:128, 0:64] = wk(a, b)    # x1(shifted) -> out0, tap (a,b)
        if (a, b) == (0, 0):
            blk[0:64, 0:64] = wc        # x0 -> out0 center
        if (a, b) == (1, 1):
            blk[64:128, 64:128] = wc    # x1(shifted) -> out1 center
    return wp


def kernel(x0, x1, g1, b1, w1_center, w1_corner, bias1,
           g2, b2, w2_center, w2_corner, bias2,
           w_skip, bias_skip, g_skip, beta_skip):
    x0 = np.asarray(x0, np.float32)
    x1 = np.asarray(x1, np.float32)

    # ---- constants ----
    cbf = np.zeros((128, 1408), np.float32)
    cbf[:, 0:512] = _pack_weights(np.asarray(w1_center), np.asarray(w1_corner))
    cbf[:, 512:1024] = _pack_weights(np.asarray(w2_center), np.asarray(w2_corner))
    wskf = np.zeros((128, 128), np.float32)
    wskf[0:64, 0:64] = np.asarray(w_skip).T
    wskf[64:128, 64:128] = np.asarray(w_skip).T
    cbf[:, 1024:1152] = wskf
    cbf[:, 1152:1280] = np.eye(128, dtype=np.float32)
    gind = np.zeros((128, 8), np.float32)
    for p in range(128):
        gind[p, (p % 64) // 8] = 1.0
    gfull = gind @ gind.T
    cbf[:, 1280:1408] = gfull
    cbf = cbf.astype(BF16)

    pp2 = lambda v: np.concatenate([np.asarray(v, np.float32)] * 2)
    base_cf = np.zeros((128, 16), np.float32)
    for i, v in enumerate((g1, b1, bias1, g2, b2, bias2,
                           g_skip, beta_skip, bias_skip)):
        base_cf[:, i] = pp2(v)
    base_cf[:, 9:13] = 1.0  # masks default

    # ---- per-core inputs ----
    in_maps = []
    for core in range(NCORES):
        b, k = core // 4, core % 4
        r0 = RPC * k
        xp = np.zeros((128, RPC, W), np.float32)
        xp[0:64] = x0[b, :, r0:r0 + RPC, :]
        xp[64:128] = x1[b, :, r0:r0 + RPC, :]

        x2 = np.zeros((128, XR, WP), np.float32)
        # x0 rows R0-2 .. R1+2 at storage rows 0..67, cols 2:258
        lo, hi = r0 - 2, r0 + RPC + 2
        vlo, vhi = max(0, lo), min(H, hi)
        x2[0:64, vlo - lo:vhi - lo, 2:258] = x0[b, :, vlo:vhi, :]
        # x1 rows R0-3 .. R1+1 at storage rows 0..67, cols 3:259
        lo1, hi1 = r0 - 3, r0 + RPC + 1
        v1lo, v1hi = max(0, lo1), min(H, hi1)
        x2[64:128, v1lo - lo1:v1hi - lo1, 3:259] = x1[b, :, v1lo:v1hi, :]

        cf32 = base_cf.copy()
        if k == 0:
            cf32[64:128, 9] = 0.0    # mxa: X2 row 2 (x1 row -1)
            cf32[64:128, 11] = 0.0   # mha: H2 row 1 (h1 row -1)
        if k == 3:
            cf32[0:64, 10] = 0.0     # mxb: X2 row 66 (x0 row H)
            cf32[0:64, 12] = 0.0     # mhb: H2 row 65 (h0 row H)

        in_maps.append({
            "xp": xp.astype(BF16), "x2": x2.astype(BF16),
            "cbf": cbf, "cf32": cf32, "gfull": gfull,
        })

    nc = _get_nc()
    _CACHE["in_maps"] = in_maps
    res = run_bass_kernel_spmd(nc, in_maps, list(range(NCORES)))
    _CACHE["last_results"] = res

    out = np.empty((2, B, C, H, W), np.float32)
    for core in range(NCORES):
        b, k = core // 4, core % 4
        r0 = RPC * k
        arr = np.asarray(res.results[core]["out"]).astype(np.float32)
        out[0, b, :, r0:r0 + RPC, :] = arr[0:64]
        out[1, b, :, r0:r0 + RPC, :] = arr[64:128]
    return out


# revision 6
# speedup vs baseline: 1.1330x; 1.0201x over previous
"""Quincunx ConvBlock (GN->ReLU->qcConv x2 + skip 1x1 conv + GN, residual add)
on 8 TRN2 NeuronCores.

Sharding: batch (2) x H-quarters (4) -> 8 cores. Each core owns 64 output rows
of both cosets. GroupNorm stats need global (batch, group) sums -> two small
AllReduces over core groups {0..3} / {4..7} (GN1; skip-GN + GN2 merged).

SBUF layout: 128 partitions = [coset0 ch 0..63 | coset1 ch 0..63].
Conv tiles store coset1 pre-shifted by (+1,+1) so each 2-row conv block is
4 full K=128 matmuls (one per 2x2 tap shift); the center taps fold into the
(0,0)/(1,1) shift weights. Plain-layout tiles feed stats and the skip conv.

PSUM drains run on the Scalar (ACT) engine with sum-accumulators harvesting
GN stats; square-sums via DVE scalar_tensor_tensor. Applies (ReLU) interleave
with conv matmul groups so ACT and PE overlap.
"""
import numpy as np
import ml_dtypes

import concourse.bass as bass
import concourse.tile as tile
from concourse import bacc, mybir
from concourse.bass_utils import run_bass_kernel_spmd

BF16 = ml_dtypes.bfloat16

B, C, H, W = 2, 64, 256, 256
G = 8
EPS = 1e-5
NCORES = 8
RPC = H // 4     # 64 owned rows per core
XR = RPC + 4     # conv x tile rows
HR = RPC + 2     # h tile rows (h rows R0-1 .. R1+1)
WP = W + 4       # padded row width for conv tiles

DT = mybir.dt.float32
BF = mybir.dt.bfloat16
AF = mybir.ActivationFunctionType
ALU = mybir.AluOpType

SHIFTS = ((0, 0), (0, 1), (1, 0), (1, 1))

_CACHE = {}


def _rsqrt_newton(nc, out, v_ap, tmps):
    """out = rsqrt(v + EPS) via ACT sqrt + DVE recip + 1 Newton step."""
    ve, sd, y0, t, u, w = (tmps[k] for k in ("ve", "sd", "y0", "t", "u", "w"))
    nc.vector.tensor_scalar(ve[:], v_ap, EPS, None, ALU.add)
    nc.scalar.activation(sd[:], ve[:], AF.Sqrt)
    nc.vector.reciprocal(y0[:], sd[:])
    nc.vector.tensor_tensor(t[:], ve[:], y0[:], ALU.mult)
    nc.vector.tensor_tensor(u[:], t[:], y0[:], ALU.mult)
    nc.vector.tensor_scalar(w[:], u[:], -0.5, 1.5, ALU.mult, ALU.add)
    nc.vector.tensor_tensor(out, y0[:], w[:], ALU.mult)


def _gn_coeffs(nc, a, c, mean_ap, e2_ap, g_ap, b_ap, tmps):
    """a = g*rsqrt(var+eps), c = b - mean*a  from (mean, E[x^2])."""
    nc.vector.tensor_tensor(tmps["m"][:], mean_ap, mean_ap, ALU.mult)
    nc.vector.tensor_tensor(tmps["mm"][:], e2_ap, tmps["m"][:], ALU.subtract)
    _rsqrt_newton(nc, tmps["sd"][:], tmps["mm"][:], tmps)
    nc.vector.tensor_tensor(a[:], tmps["sd"][:], g_ap, ALU.mult)
    nc.vector.tensor_tensor(tmps["m"][:], mean_ap, a[:], ALU.mult)
    nc.vector.tensor_tensor(c[:], b_ap, tmps["m"][:], ALU.subtract)


def _build():
    nc = bacc.Bacc("TRN2", target_bir_lowering=False, debug=False,
                   num_devices=NCORES)

    xp_d = nc.dram_tensor("xp", [128, RPC, W], BF, kind="ExternalInput")
    x2_d = nc.dram_tensor("x2", [128, XR, WP], BF, kind="ExternalInput")
    cb_d = nc.dram_tensor("cbf", [128, 1408], BF, kind="ExternalInput")
    cf_d = nc.dram_tensor("cf32", [128, 16], DT, kind="ExternalInput")
    gt_d = nc.dram_tensor("gfull", [128, 128], DT, kind="ExternalInput")
    out_d = nc.dram_tensor("out", [128, RPC, W], BF, kind="ExternalOutput")

    with tile.TileContext(nc) as tc:
        with (
            tc.tile_pool(name="big", bufs=1) as big,
            tc.tile_pool(name="consts", bufs=1) as cpool,
            tc.tile_pool(name="stats", bufs=1) as spool,
            tc.tile_pool(name="trash", bufs=2) as tpool,
            tc.tile_pool(name="psum", bufs=2, space="PSUM") as pp,
            tc.tile_pool(name="psmall", bufs=1, space="PSUM") as ps,
            tc.tile_pool(name="dram", bufs=1, space="DRAM") as dp,
        ):
            X2 = big.tile([128, XR, WP], BF, tag="X2")
            XO = big.tile([128, RPC, W], BF, tag="XO")   # plain x, later OUT
            Hs = big.tile([128, HR, W], BF, tag="Hs")    # unshifted h
            H2 = big.tile([128, HR, WP], BF, tag="H2")   # conv-layout h
            S = big.tile([128, RPC, W], BF, tag="S")

            cb = cpool.tile([128, 1408], BF)
            cf = cpool.tile([128, 16], DT)
            gt = cpool.tile([128, 128], DT)

            w1 = cb[:, 0:512]
            w2 = cb[:, 512:1024]
            wsk = cb[:, 1024:1152]
            ident = cb[:, 1152:1280]
            gind_bf = cb[:, 1280:1408]
            pp_g1, pp_b1, pp_bias1 = cf[:, 0:1], cf[:, 1:2], cf[:, 2:3]
            pp_g2, pp_b2, pp_bias2 = cf[:, 3:4], cf[:, 4:5], cf[:, 5:6]
            pp_gs, pp_bs, pp_biass = cf[:, 6:7], cf[:, 7:8], cf[:, 8:9]
            mxa, mxb = cf[:, 9:10], cf[:, 10:11]
            mha, mhb = cf[:, 11:12], cf[:, 12:13]

            # dummy collective: absorbs CC barrier / core-start skew
            dmy = spool.tile([8, 1], DT, tag="dmy", name="dmy")
            dmy_i = dp.tile([8, 1], DT, tag="dmyi", name="dmy_i")
            dmy_o = dp.tile([8, 1], DT, tag="dmyo", name="dmy_o")
            nc.vector.memset(dmy[:], 0.0)
            nc.sync.dma_start(dmy_i[:], dmy[:])
            RG = [[0, 1, 2, 3], [4, 5, 6, 7]]
            nc.gpsimd.collective_compute("AllReduce", ALU.add,
                                         replica_groups=RG,
                                         ins=[dmy_i.opt()], outs=[dmy_o.opt()])

            # ---------------- DMAs in ----------------
            nc.sync.dma_start(cb[:], cb_d[:])
            nc.sync.dma_start(cf[:], cf_d[:])
            nc.sync.dma_start(gt[:], gt_d[:])
            for c in range(8):
                nc.sync.dma_start(XO[:, 8 * c:8 * (c + 1), :],
                                  xp_d[:, 8 * c:8 * (c + 1), :])
            for c in range(4):
                nc.sync.dma_start(X2[:, 17 * c:17 * (c + 1), :],
                                  x2_d[:, 17 * c:17 * (c + 1), :])

            def st(name, shape, dt=DT):
                return spool.tile(shape, dt, tag=name, name=name)

            xsq = st("xsq", [128, 8])
            sdr = st("sdr", [128, 11])
            ssq = st("ssq", [128, 11])
            hdr = st("hdr", [128, 11])
            hsq = st("hsq", [128, 11])
            xm2 = st("xm2", [128, 1])
            stx = st("stx", [128, 2])
            st4 = st("st4", [128, 4])
            s4r = st("s4r", [128, 4])
            g1b = st("g1b", [128, 2])
            g4b = st("g4b", [128, 4])
            a1, c1 = st("a1", [128, 1]), st("c1", [128, 1])
            a2, c2 = st("a2", [128, 1]), st("c2", [128, 1])
            asv, csv = st("asv", [128, 1]), st("csv", [128, 1])
            dcon = st("dcon", [128, 1])
            tmps = {k: st("nt_" + k, [128, 1])
                    for k in ("ve", "sd", "y0", "t", "u", "w", "m", "mm")}

            ccx_i = dp.tile([128, 2], DT, tag="ccxi", name="ccx_i")
            ccx_o = dp.tile([128, 2], DT, tag="ccxo", name="ccx_o")
            cc4_i = dp.tile([128, 4], DT, tag="cc4i", name="cc4_i")
            cc4_o = dp.tile([128, 4], DT, tag="cc4o", name="cc4_o")

            # H2 pad columns (never written by drain copies) must be zero
            nc.gpsimd.memset(H2[0:64, :, 258:259], 0.0)
            nc.gpsimd.memset(H2[64:128, :, 2:3], 0.0)

            # ---------------- GN1 stats (ACT squares + PE sums) -------------
            for j in range(8):
                tr = tpool.tile([128, 8, 256], BF, tag="trash")
                nc.scalar.activation(tr[:], XO[:, 8 * j:8 * (j + 1), :],
                                     AF.Square, accum_out=xsq[:, j:j + 1])
            pm1 = ps.tile([128, 512], DT, tag="pm1")
            for t in range(32):
                nc.tensor.matmul(pm1[:, :], gind_bf[:],
                                 XO[:, 2 * t:2 * t + 2, :],
                                 start=(t == 0), stop=(t == 31),
                                 skip_group_check=True)

            # ---------------- AR1 ----------------
            nc.vector.reduce_sum(xm2[:], xsq[:], axis=mybir.AxisListType.X)
            pstat = ps.tile([128, 512], DT, tag="pstat", name="pstat")
            nc.tensor.matmul(pstat[:, 0:1], gt[:], xm2[:], start=True, stop=True)
            nc.vector.reduce_sum(stx[:, 0:1], pm1[:, :], axis=mybir.AxisListType.X)
            nc.vector.tensor_copy(stx[:, 1:2], pstat[:, 0:1])
            nc.sync.dma_start(ccx_i[:], stx[:])
            nc.gpsimd.collective_compute("AllReduce", ALU.add, replica_groups=RG,
                                         ins=[ccx_i.opt()], outs=[ccx_o.opt()])
            nc.sync.dma_start(stx[:], ccx_o[:])

            # ---------------- skip conv + S drains (DVE) + ssq --------------
            sk_groups = [3] * 10 + [2]
            bi = 0
            for g, nb in enumerate(sk_groups):
                pg = pp.tile([128, 1536], DT, tag="pg")
                for k in range(nb):
                    t = bi + k
                    nc.tensor.matmul(pg[:, 512 * k:512 * (k + 1)], wsk[:],
                                     XO[:, 2 * t:2 * t + 2, :],
                                     start=True, stop=True)
                nc.vector.tensor_scalar(S[:, 2 * bi:2 * (bi + nb), :],
                                        pg[:, 0:512 * nb], pp_biass, 0.0, ALU.add,
                                        ALU.add, accum_out=sdr[:, g:g + 1])
                trs = tpool.tile([128, 6, 256], BF, tag="trs", name="trs")
                nc.vector.scalar_tensor_tensor(
                    trs[:, 0:nb * 2, :], S[:, 2 * bi:2 * (bi + nb), :], 1.0,
                    S[:, 2 * bi:2 * (bi + nb), :], ALU.mult, ALU.mult,
                    accum_out=ssq[:, g:g + 1])
                bi += nb

            # GN1 coeffs (DVE) after all drains so the queue never stalls
            nc.vector.tensor_scalar(g1b[:], stx[:], 1.0 / (1 << 20),
                                    None, ALU.mult)
            _gn_coeffs(nc, a1, c1, g1b[:, 0:1], g1b[:, 1:2],
                       pp_g1, pp_b1, tmps)

            # ---------------- apply1 on X2 (ACT) ----------------------------
            # slivers: the one data column outside the shared range per half
            nc.scalar.activation(X2[0:64, :, 2:3], X2[0:64, :, 2:3],
                                 AF.Relu, bias=c1[0:64], scale=a1[0:64])
            nc.scalar.activation(X2[64:128, :, 258:259], X2[64:128, :, 258:259],
                                 AF.Relu, bias=c1[64:128], scale=a1[64:128])

            a1_chunks = [(0, 12), (12, 24), (24, 36), (36, 48), (48, 60), (60, 68)]

            def emit_apply1(c):
                r0, r1 = a1_chunks[c]
                nc.scalar.activation(X2[:, r0:r1, 3:258], X2[:, r0:r1, 3:258],
                                     AF.Relu, bias=c1[:], scale=a1[:])
                if c == 0:  # row 2 mask (x1 row -1 on core 0)
                    nc.vector.tensor_scalar(X2[:, 2:3, 2:259], X2[:, 2:3, 2:259],
                                            mxa, None, ALU.mult)
                if c == 5:  # row 66 mask (x0 row H on core 3)
                    nc.vector.tensor_scalar(X2[:, 66:67, 2:259], X2[:, 66:67, 2:259],
                                            mxb, None, ALU.mult)

            emit_apply1(0)
            emit_apply1(1)

            # ---------------- conv1 + drains (ACT) + H2 DMAs + hsq ----------
            def conv_block(pg, off, T, r0, wpk, extra=None):
                for si, (dr, dc) in enumerate(SHIFTS):
                    nc.tensor.matmul(pg[:, off:off + 512],
                                     wpk[:, 128 * si:128 * (si + 1)],
                                     T[:, r0 + dr:r0 + dr + 2, 2 + dc:2 + dc + 256],
                                     start=(si == 0),
                                     stop=(si == 3 and extra is None))
                if extra is not None:
                    lhs, rhs = extra
                    nc.tensor.matmul(pg[:, off:off + 512], lhs, rhs,
                                     start=False, stop=True)

            napply = 2
            for g in range(11):
                pg = pp.tile([128, 1536], DT, tag="pg")
                for k in range(3):
                    t = 3 * g + k
                    conv_block(pg, 512 * k, X2, 1 + 2 * t, w1)
                # drain rows 6g..6g+5 to Hs on ACT; accumulate sums over
                # owned h rows only (exclude storage rows 0 and 65)
                if g == 0:
                    nc.scalar.activation(Hs[:, 0:1, :], pg[:, 0:256],
                                         AF.Identity, bias=pp_bias1)
                    nc.scalar.activation(Hs[:, 1:6, :], pg[:, 256:1536],
                                         AF.Identity, bias=pp_bias1,
                                         accum_out=hdr[:, 0:1])
                elif g == 10:
                    nc.scalar.activation(Hs[:, 60:65, :], pg[:, 0:1280],
                                         AF.Identity, bias=pp_bias1,
                                         accum_out=hdr[:, 10:11])
                    nc.scalar.activation(Hs[:, 65:66, :], pg[:, 1280:1536],
                                         AF.Identity, bias=pp_bias1)
                else:
                    nc.scalar.activation(Hs[:, 6 * g:6 * g + 6, :], pg[:, 0:1536],
                                         AF.Identity, bias=pp_bias1,
                                         accum_out=hdr[:, g:g + 1])
                # interleave remaining apply1 chunks between drains
                if napply < 6:
                    emit_apply1(napply)
                    napply += 1
                # conv-layout copies (DMA): h0 straight, h1 shifted (+1,+1)
                nc.sync.dma_start(H2[0:64, 6 * g:6 * g + 6, 2:258],
                                  Hs[0:64, 6 * g:6 * g + 6, :])
                if g < 10:
                    nc.sync.dma_start(H2[64:128, 6 * g + 1:6 * g + 7, 3:259],
                                      Hs[64:128, 6 * g:6 * g + 6, :])
                else:
                    nc.sync.dma_start(H2[64:128, 61:66, 3:259],
                                      Hs[64:128, 60:65, :])
                # sum of h^2 over owned rows (DVE)
                ra, rb = (1, 6) if g == 0 else (6 * g, min(6 * g + 6, 65))
                trh = tpool.tile([128, 6, 256], BF, tag="trh", name="trh")
                nc.vector.scalar_tensor_tensor(
                    trh[:, 0:rb - ra, :], Hs[:, ra:rb, :], 1.0,
                    Hs[:, ra:rb, :], ALU.mult, ALU.mult,
                    accum_out=hsq[:, g:g + 1])

            # ---------------- AR2 (skip-GN + GN2 merged) --------------------
            nc.vector.reduce_sum(st4[:, 0:1], sdr[:], axis=mybir.AxisListType.X)
            nc.vector.reduce_sum(st4[:, 1:2], ssq[:], axis=mybir.AxisListType.X)
            nc.vector.reduce_sum(st4[:, 2:3], hdr[:], axis=mybir.AxisListType.X)
            nc.vector.reduce_sum(st4[:, 3:4], hsq[:], axis=mybir.AxisListType.X)
            nc.tensor.matmul(pstat[:, 8:12], gt[:], st4[:], start=True, stop=True)
            nc.vector.tensor_copy(s4r[:], pstat[:, 8:12])
            nc.sync.dma_start(cc4_i[:], s4r[:])
            nc.gpsimd.collective_compute("AllReduce", ALU.add, replica_groups=RG,
                                         ins=[cc4_i.opt()], outs=[cc4_o.opt()])
            nc.sync.dma_start(s4r[:], cc4_o[:])
            nc.vector.tensor_scalar(g4b[:], s4r[:], 1.0 / (1 << 20), None, ALU.mult)
            _gn_coeffs(nc, asv, csv, g4b[:, 0:1], g4b[:, 1:2], pp_gs, pp_bs, tmps)
            _gn_coeffs(nc, a2, c2, g4b[:, 2:3], g4b[:, 3:4], pp_g2, pp_b2, tmps)
            nc.vector.tensor_tensor(dcon[:], pp_bias2, csv[:], ALU.add)
            # prescale S by a_s in-place (residual folds into conv2 drain);
            # first chunk now, the rest interleave with the drains below
            npre = 0

            def emit_prescale(upto_row):
                nonlocal npre
                while 16 * npre < upto_row:
                    nc.vector.tensor_scalar(S[:, 16 * npre:16 * (npre + 1), :],
                                            S[:, 16 * npre:16 * (npre + 1), :],
                                            asv[:], None, ALU.mult)
                    npre += 1

            emit_prescale(16)

            # ---------------- apply2 on H2 (ACT) ----------------------------
            nc.scalar.activation(H2[0:64, :, 2:3], H2[0:64, :, 2:3],
                                 AF.Relu, bias=c2[0:64], scale=a2[0:64])
            nc.scalar.activation(H2[64:128, :, 258:259], H2[64:128, :, 258:259],
                                 AF.Relu, bias=c2[64:128], scale=a2[64:128])

            a2_chunks = [(0, 11), (11, 22), (22, 33), (33, 44), (44, 55), (55, 66)]

            def emit_apply2(c):
                r0, r1 = a2_chunks[c]
                nc.scalar.activation(H2[:, r0:r1, 3:258], H2[:, r0:r1, 3:258],
                                     AF.Relu, bias=c2[:], scale=a2[:])
                if c == 0:  # row 1 mask (h1 row -1 on core 0)
                    nc.vector.tensor_scalar(H2[:, 1:2, 2:259], H2[:, 1:2, 2:259],
                                            mha, None, ALU.mult)
                if c == 5:  # row 65 mask (h0 row H on core 3)
                    nc.vector.tensor_scalar(H2[:, 65:66, 2:259], H2[:, 65:66, 2:259],
                                            mhb, None, ALU.mult)

            emit_apply2(0)
            emit_apply2(1)

            # ---------------- conv2 + fused residual drain + DMA out --------
            c2_groups = [3] * 10 + [2]
            bi = 0
            napply = 2
            for g, nb in enumerate(c2_groups):
                pg = pp.tile([128, 1536], DT, tag="pg")
                for k in range(nb):
                    t = bi + k
                    conv_block(pg, 512 * k, H2, 1 + 2 * t, w2)
                emit_prescale(2 * (bi + nb))
                # OUT = conv2psum + dcon + a_s*S   (DVE, one pass)
                nc.vector.scalar_tensor_tensor(
                    XO[:, 2 * bi:2 * (bi + nb), :], pg[:, 0:512 * nb], dcon[:],
                    S[:, 2 * bi:2 * (bi + nb), :], ALU.add, ALU.add)
                if napply < 6:
                    emit_apply2(napply)
                    napply += 1
                nc.sync.dma_start(out_d[:, 2 * bi:2 * (bi + nb), :],
                                  XO[:, 2 * bi:2 * (bi + nb), :])
                bi += nb

    nc.compile()
    return nc


def _get_nc():
    if "nc" not in _CACHE:
        _CACHE["nc"] = _build()
    return _CACHE["nc"]


# --------------------------------------------------------------------------
# host side
# --------------------------------------------------------------------------
def _pack_weights(w_center, w_corner):
    """[128, 512] packed lhsT weights: 4 shift blocks of [128,128]."""
    wp = np.zeros((128, 512), np.float32)
    wc = w_center.T
    wk = lambda a, b: w_corner[:, :, a, b].T
    for si, (a, b) in enumerate(SHIFTS):
        blk = wp[:, 128 * si:128 * (si + 1)]
        blk[0:64, 64:128] = wk(a, b)    # x0 -> out1, tap (a,b)
        blk[64:128, 0:64] = wk(a, b)    # x1(shifted) -> out0, tap (a,b)
        if (a, b) == (0, 0):
            blk[0:64, 0:64] = wc        # x0 -> out0 center
        if (a, b) == (1, 1):
            blk[64:128, 64:128] = wc    # x1(shifted) -> out1 center
    return wp


def kernel(x0, x1, g1, b1, w1_center, w1_corner, bias1,
           g2, b2, w2_center, w2_corner, bias2,
           w_skip, bias_skip, g_skip, beta_skip):
    x0 = np.asarray(x0, np.float32)
    x1 = np.asarray(x1, np.float32)

    # ---- constants ----
    cbf = np.zeros((128, 1408), np.float32)
    cbf[:, 0:512] = _pack_weights(np.asarray(w1_center), np.asarray(w1_corner))
    cbf[:, 512:1024] = _pack_weights(np.asarray(w2_center), np.asarray(w2_corner))
    wskf = np.zeros((128, 128), np.float32)
    wskf[0:64, 0:64] = np.asarray(w_skip).T
    wskf[64:128, 64:128] = np.asarray(w_skip).T
    cbf[:, 1024:1152] = wskf
    cbf[:, 1152:1280] = np.eye(128, dtype=np.float32)
    gind = np.zeros((128, 8), np.float32)
    for p in range(128):
        gind[p, (p % 64) // 8] = 1.0
    gfull = gind @ gind.T
    cbf[:, 1280:1408] = gfull
    cbf = cbf.astype(BF16)

    pp2 = lambda v: np.concatenate([np.asarray(v, np.float32)] * 2)
    base_cf = np.zeros((128, 16), np.float32)
    for i, v in enumerate((g1, b1, bias1, g2, b2, bias2,
                           g_skip, beta_skip, bias_skip)):
        base_cf[:, i] = pp2(v)
    base_cf[:, 9:13] = 1.0  # masks default

    # ---- per-core inputs ----
    in_maps = []
    for core in range(NCORES):
        b, k = core // 4, core % 4
        r0 = RPC * k
        xp = np.zeros((128, RPC, W), np.float32)
        xp[0:64] = x0[b, :, r0:r0 + RPC, :]
        xp[64:128] = x1[b, :, r0:r0 + RPC, :]

        x2 = np.zeros((128, XR, WP), np.float32)
        # x0 rows R0-2 .. R1+2 at storage rows 0..67, cols 2:258
        lo, hi = r0 - 2, r0 + RPC + 2
        vlo, vhi = max(0, lo), min(H, hi)
        x2[0:64, vlo - lo:vhi - lo, 2:258] = x0[b, :, vlo:vhi, :]
        # x1 rows R0-3 .. R1+1 at storage rows 0..67, cols 3:259
        lo1, hi1 = r0 - 3, r0 + RPC + 1
        v1lo, v1hi = max(0, lo1), min(H, hi1)
        x2[64:128, v1lo - lo1:v1hi - lo1, 3:259] = x1[b, :, v1lo:v1hi, :]

        cf32 = base_cf.copy()
        if k == 0:
            cf32[64:128, 9] = 0.0    # mxa: X2 row 2 (x1 row -1)
            cf32[64:128, 11] = 0.0   # mha: H2 row 1 (h1 row -1)
        if k == 3:
            cf32[0:64, 10] = 0.0     # mxb: X2 row 66 (x0 row H)
            cf32[0:64, 12] = 0.0     # mhb: H2 row 65 (h0 row H)

        in_maps.append({
            "xp": xp.astype(BF16), "x2": x2.astype(BF16),
            "cbf": cbf, "cf32": cf32, "gfull": gfull,
        })

    nc = _get_nc()
    _CACHE["in_maps"] = in_maps
    res = run_bass_kernel_spmd(nc, in_maps, list(range(NCORES)))
    _CACHE["last_results"] = res

    out = np.empty((2, B, C, H, W), np.float32)
    for core in range(NCORES):
        b, k = core // 4, core % 4
        r0 = RPC * k
        arr = np.asarray(res.results[core]["out"]).astype(np.float32)
        out[0, b, :, r0:r0 + RPC, :] = arr[0:64]
        out[1, b, :, r0:r0 + RPC, :] = arr[64:128]
    return out
